# revision 1
# baseline (speedup 1.0000x reference)
"""Trainium2 Bass kernel for CausalWanGameActionTransformerBlock (dual-branch
frame-causal attention: RoPE branch + PRoPE branch).

Sharding: 24 (branch, head) units across 8 cores, 3 units per core.
Cores 0-3: RoPE heads 0-11. Cores 4-7: PRoPE heads 0-11.

Device computes, per unit: scores^T = K^T-tiles @ Q (fp32r matmuls),
exp via ScalarE (scale folded into activation), PV + replicated row-sum
via PE, normalization on DVE. Host applies the branch pre/post transforms
(RoPE rotation / PRoPE projective maps) and all layout packing.
"""

import numpy as np

# ---- problem constants (hardcoded; kernel.py must be self-contained) ----
B = 1
F = 3
T = 880            # tokens per frame
S = F * T          # 2640
NH = 12
HD = 128
SCALE = 1.0 / np.sqrt(HD)
N_CORES = 8
UPC = 3            # units per core; unit u = core*3 + j; branch = u // NH, head = u % NH
NKT = (S + 127) // 128   # 21 key tiles over the full sequence
HALF_SPLITS = ((0, 512), (512, 368))  # psum-bank-aligned free-dim splits of T=880


# ------------------------------------------------------------------
# device program
# ------------------------------------------------------------------

def build_program(reps: int = 1, loop_n: int = 1):
    """Build the per-core Bass program (identical on all cores).

    reps > 1 unrolls the whole body; loop_n > 1 wraps it in a hardware
    For_i loop (used for marginal-cost timing in test.py — the body runs
    loop_n times per execution with only ~2us of back-edge overhead).
    """
    from contextlib import ExitStack
    import concourse.tile as tile
    from concourse import bacc, mybir

    f32 = mybir.dt.float32
    f32r = mybir.dt.float32r
    Exp = mybir.ActivationFunctionType.Exp

    nc = bacc.Bacc("TRN2", target_bir_lowering=False, debug=False,
                   num_devices=N_CORES)

    qT_d = nc.dram_tensor("qT", [UPC, HD, S], f32r, kind="ExternalInput").ap()
    kT_d = nc.dram_tensor("kT", [UPC, HD, S], f32r, kind="ExternalInput").ap()
    v_d = nc.dram_tensor("v", [UPC, 128, NKT, HD], f32r, kind="ExternalInput").ap()
    ones_d = nc.dram_tensor("ones", [128, 128], f32r, kind="ExternalInput").ap()
    o_d = nc.dram_tensor("o", [UPC, HD, S], f32, kind="ExternalOutput").ap()

    # L-work split: which engine accumulates each k-tile's row-sum partial.
    # DVE adds into an SBUF partial (1.04us/tile); PE takes a ~3/8 share as
    # ones-matmuls into lacc (0.37us/tile) using its headroom under the
    # ACT exp wall. GPSIMD proved slower on HW than modeled.
    LPAT = ("dve", "pe", "dve", "dve", "pe", "dve", "dve", "pe")

    with ExitStack() as ctx:
        tc = ctx.enter_context(tile.TileContext(nc))
        const = ctx.enter_context(tc.tile_pool(name="const", bufs=1))
        big = ctx.enter_context(tc.tile_pool(name="big", bufs=2))
        pexp = ctx.enter_context(tc.tile_pool(name="pexp", bufs=6))
        lsum = ctx.enter_context(tc.tile_pool(name="lsum", bufs=2))
        outs = ctx.enter_context(tc.tile_pool(name="outs", bufs=2))
        psum = ctx.enter_context(tc.tile_pool(name="psum", bufs=1, space="PSUM"))

        ones = const.tile([128, 128], f32r)
        nc.sync.dma_start(ones, ones_d)

        loop_ctx = tc.For_i(0, loop_n, 1) if loop_n > 1 else None
        if loop_ctx is not None:
            ctx.enter_context(loop_ctx)

        for _rep in range(reps):
            for u in range(UPC):
                qs = big.tile([HD, S], f32r, tag="q")
                ks = big.tile([HD, S], f32r, tag="k")
                vs = big.tile([128, NKT, HD], f32r, tag="v")
                # frame-sliced loads so frame-0 compute starts early
                for f in range(F):
                    cols = slice(f * T, (f + 1) * T)
                    nc.sync.dma_start(qs[:, cols], qT_d[u][:, cols])
                    nc.sync.dma_start(ks[:, cols], kT_d[u][:, cols])
                    c0, c1 = f * 7, min(NKT, (f + 1) * 7)
                    nc.sync.dma_start(vs[:, c0:c1], v_d[u][:, c0:c1])

                for f in range(F):
                    kv = (f + 1) * T        # visible key prefix
                    nkt = (kv + 127) // 128
                    acc = psum.tile([128, T], f32, tag="acc")
                    lacc = psum.tile([128, T], f32, tag="lacc")
                    ldve = lsum.tile([128, T], f32r, tag="ldve")
                    lacc_started = [False]

                    def qk(kt):
                        k0 = kt * 128
                        ksz = min(128, kv - k0)
                        ps = psum.tile([128, T], f32, tag="ps", bufs=2,
                                       name=f"ps_{u}_{f}_{kt}")
                        for (h0, hw) in HALF_SPLITS:
                            nc.tensor.matmul(
                                ps[:ksz, h0:h0 + hw],
                                lhsT=ks[:, k0:k0 + ksz],
                                rhs=qs[:, f * T + h0:f * T + h0 + hw],
                                start=True, stop=True)
                        pe = pexp.tile([128, T], f32r, tag="pe",
                                       name=f"pe_{u}_{f}_{kt}")
                        nc.scalar.activation(pe[:ksz], ps[:ksz], Exp, scale=SCALE)
                        return pe

                    def pv(kt, pe):
                        ksz = min(128, kv - kt * 128)
                        first, last = kt == 0, kt == nkt - 1
                        for (h0, hw) in HALF_SPLITS:
                            nc.tensor.matmul(
                                acc[:, h0:h0 + hw],
                                lhsT=vs[:ksz, kt, :],
                                rhs=pe[:ksz, h0:h0 + hw],
                                start=first, stop=last)
                        # row-sum partial: DVE accumulates in SBUF; PE's
                        # share goes straight into lacc as ones-matmuls
                        if kt == 0:
                            nc.vector.tensor_copy(ldve[:ksz], pe[:ksz])
                        elif LPAT[kt % len(LPAT)] == "dve":
                            nc.vector.tensor_add(ldve[:ksz], ldve[:ksz], pe[:ksz])
                        else:
                            st = not lacc_started[0]
                            lacc_started[0] = True
                            for (h0, hw) in HALF_SPLITS:
                                nc.tensor.matmul(
                                    lacc[:, h0:h0 + hw],
                                    lhsT=ones[:ksz, :],
                                    rhs=pe[:ksz, h0:h0 + hw],
                                    start=st, stop=False)

                    # software pipeline: QK runs one k-tile ahead of PV
                    pe_prev = qk(0)
                    for kt in range(1, nkt):
                        pe_cur = qk(kt)
                        pv(kt - 1, pe_prev)
                        pe_prev = pe_cur
                    pv(nkt - 1, pe_prev)

                    # column sum of the DVE partial -> lacc (replicated)
                    for (h0, hw) in HALF_SPLITS:
                        nc.tensor.matmul(
                            lacc[:, h0:h0 + hw],
                            lhsT=ones,
                            rhs=ldve[:, h0:h0 + hw],
                            start=not lacc_started[0], stop=True)

                    rec = outs.tile([128, T], f32, tag="rec")
                    nc.vector.reciprocal(rec, lacc)
                    o = outs.tile([128, T], f32, tag="o")
                    nc.vector.tensor_mul(o, acc, rec)
                    nc.sync.dma_start(o_d[u][:, f * T:(f + 1) * T], o)

    nc.compile()
    return nc


# ------------------------------------------------------------------
# host-side transforms + packing
# ------------------------------------------------------------------

def _rope_rotate(x, cos, sin):
    # x: [S, HD]; cos/sin: [S, HD//2]; interleaved pairs (is_neox_style=False)
    o = np.empty_like(x)
    x1 = x[:, 0::2]
    x2 = x[:, 1::2]
    o[:, 0::2] = x1 * cos - x2 * sin
    o[:, 1::2] = x2 * cos + x1 * sin
    return o


def _prope_mats(viewmats, Ks):
    # P_f = pad(K_f) @ viewmat_f ; returns P, Pinv as [F,4,4] fp64 for accuracy
    P = np.zeros((F, 4, 4), np.float64)
    for f in range(F):
        Kpad = np.zeros((4, 4), np.float64)
        Kpad[:3, :3] = Ks[0, f].astype(np.float64)
        Kpad[3, 3] = 1.0
        P[f] = Kpad @ viewmats[0, f].astype(np.float64)
    Pinv = np.linalg.inv(P)
    return P, Pinv


def _apply_44(x, mats):
    # x: [S, HD] -> per-frame f: chunk [T, 32, 4] @ mats[f] (right-multiply)
    o = np.empty_like(x)
    xc = x.reshape(F, T, HD // 4, 4)
    oc = o.reshape(F, T, HD // 4, 4)
    for f in range(F):
        oc[f] = (xc[f].astype(np.float64) @ mats[f]).astype(np.float32)
    return o


def _pack_inputs(q, k, v, cos, sin, viewmats, Ks):
    q = np.asarray(q, np.float32)
    k = np.asarray(k, np.float32)
    v = np.asarray(v, np.float32)
    cos = np.asarray(cos, np.float32)
    sin = np.asarray(sin, np.float32)
    P, Pinv = _prope_mats(np.asarray(viewmats, np.float32),
                          np.asarray(Ks, np.float32))
    PT = np.ascontiguousarray(P.transpose(0, 2, 1))       # q: chunk @ P^T
    PinvT = np.ascontiguousarray(Pinv.transpose(0, 2, 1))  # v: chunk @ Pinv^T

    in_maps = []
    for c in range(N_CORES):
        qTs, kTs, vss = [], [], []
        for j in range(UPC):
            u = c * UPC + j
            br, h = u // NH, u % NH
            qh = q[0, :, h, :]
            kh = k[0, :, h, :]
            vh = v[0, :, h, :]
            if br == 0:
                qh = _rope_rotate(qh, cos, sin)
                kh = _rope_rotate(kh, cos, sin)
            else:
                qh = _apply_44(qh, PT)
                kh = _apply_44(kh, Pinv)
                vh = _apply_44(vh, PinvT)
            qTs.append(np.ascontiguousarray(qh.T))
            kTs.append(np.ascontiguousarray(kh.T))
            vp = np.zeros((NKT * 128, HD), np.float32)
            vp[:S] = vh
            # SBUF layout [partition, chunk, d]: token = chunk*128 + partition
            vss.append(np.ascontiguousarray(
                vp.reshape(NKT, 128, HD).transpose(1, 0, 2)))
        in_maps.append({
            "qT": np.stack(qTs),
            "kT": np.stack(kTs),
            "v": np.stack(vss),
            "ones": np.ones((128, 128), np.float32),
        })
    return in_maps, P


def _unpack_outputs(results, P):
    rope = np.empty((B, S, NH, HD), np.float32)
    prope = np.empty((B, S, NH, HD), np.float32)
    PT = np.ascontiguousarray(P.transpose(0, 2, 1))
    for c in range(N_CORES):
        o = results[c]["o"]          # [UPC, HD, S]
        for j in range(UPC):
            u = c * UPC + j
            br, h = u // NH, u % NH
            oh = np.ascontiguousarray(o[j].T)   # [S, HD]
            if br == 0:
                rope[0, :, h, :] = oh
            else:
                prope[0, :, h, :] = _apply_44(oh, PT)  # out: chunk @ P^T
    return rope, prope


# ------------------------------------------------------------------
# execution
# ------------------------------------------------------------------

_PROGRAM_CACHE = {}


def _get_program(reps: int = 1, loop_n: int = 1):
    key = (reps, loop_n)
    if key not in _PROGRAM_CACHE:
        _PROGRAM_CACHE[key] = build_program(reps, loop_n)
    return _PROGRAM_CACHE[key]


def make_runner(nc):
    """Build a jit-once callable: in_maps -> list[dict] of per-core outputs.

    Mirrors concourse.bass2jax.run_bass_via_pjrt's multi-core path, but
    hoists tracing/compilation out so repeated calls can be timed.
    """
    import jax
    import numpy as _np
    from jax.sharding import Mesh, PartitionSpec
    from jax.experimental.shard_map import shard_map
    from concourse import mybir
    from concourse import bass2jax

    bass2jax.install_neuronx_cc_hook()
    assert nc.dbg_addr is None or not nc.dbg_callbacks

    partition_name = (nc.partition_id_tensor.name
                      if nc.partition_id_tensor else None)
    in_names, out_names, out_avals, zero_outs = [], [], [], []
    for alloc in nc.m.functions[0].allocations:
        if not isinstance(alloc, mybir.MemoryLocationSet):
            continue
        name = alloc.memorylocations[0].name
        if alloc.kind == "ExternalInput":
            if name != partition_name:
                in_names.append(name)
        elif alloc.kind == "ExternalOutput":
            shape = tuple(alloc.tensor_shape)
            dtype = mybir.dt.np(alloc.dtype)
            out_names.append(name)
            out_avals.append(jax.core.ShapedArray(shape, dtype))
            zero_outs.append(_np.zeros(shape, dtype))
    n_params = len(in_names)
    n_outs = len(out_avals)
    all_in_names = list(in_names) + list(out_names)
    if partition_name is not None:
        all_in_names.append(partition_name)

    def _body(*args):
        operands = list(args)
        if partition_name is not None:
            operands.append(bass2jax.partition_id_tensor())
        outs = bass2jax._bass_exec_p.bind(
            *operands,
            out_avals=tuple(out_avals),
            in_names=tuple(all_in_names),
            out_names=tuple(out_names),
            lowering_input_output_aliases=(),
            sim_require_finite=True,
            sim_require_nnan=True,
            nc=nc,
        )
        return tuple(outs)

    devices = jax.devices()[:N_CORES]
    mesh = Mesh(_np.asarray(devices), ("core",))
    in_specs = (PartitionSpec("core"),) * (n_params + n_outs)
    out_specs = (PartitionSpec("core"),) * n_outs
    # No donation: our kernel writes every output element, so stale result
    # buffers are fine and the zero "outputs" can live on device across calls.
    sharded = jax.jit(
        shard_map(_body, mesh=mesh, in_specs=in_specs, out_specs=out_specs,
                  check_rep=False),
        keep_unused=True)
    sharding = jax.sharding.NamedSharding(mesh, PartitionSpec("core"))

    def stage(in_maps):
        """Transfer per-core inputs (+zero output operands) to device once."""
        concat_in = [
            _np.concatenate([_np.asarray(in_maps[c][nm]) for c in range(N_CORES)],
                            axis=0)
            for nm in in_names
        ]
        concat_zeros = [
            _np.zeros((N_CORES * z.shape[0], *z.shape[1:]), z.dtype)
            for z in zero_outs
        ]
        return [jax.device_put(a, sharding) for a in concat_in + concat_zeros]

    def run_staged(staged, want_outputs=True):
        out_arrs = sharded(*staged)
        jax.block_until_ready(out_arrs)
        if not want_outputs:
            return None
        return [
            {nm: _np.asarray(out_arrs[i]).reshape(N_CORES, *out_avals[i].shape)[c]
             for i, nm in enumerate(out_names)}
            for c in range(N_CORES)
        ]

    def run(in_maps, want_outputs=True):
        return run_staged(stage(in_maps), want_outputs)

    run.stage = stage
    run.run_staged = run_staged
    return run


_RUNNER_CACHE = {}


def _get_runner(reps: int = 1, loop_n: int = 1):
    key = (reps, loop_n)
    if key not in _RUNNER_CACHE:
        _RUNNER_CACHE[key] = make_runner(_get_program(reps, loop_n))
    return _RUNNER_CACHE[key]


def kernel(q, k, v, cos, sin, viewmats, Ks):
    in_maps, P = _pack_inputs(q, k, v, cos, sin, viewmats, Ks)
    results = _get_runner(1)(in_maps)
    return _unpack_outputs(results, P)



# revision 2
# speedup vs baseline: 4.5865x; 4.5865x over previous
"""Trainium2 Bass kernel for CausalWanGameActionTransformerBlock.

Changes vs baseline:
- fp16 data path (q/k/v/p tiles): matmuls at 1 cyc/row, DVE element-wise
  ops hit the 2x two-byte mode, input DMA bytes halved.
- Row-sum L entirely on DVE (two interleaved fp16 accumulators, folded
  by a ones-matmul at frame end) — no streaming PE ones-matmuls, so PE
  does only QK+PV and PSUM fits acc + double-buffered score tiles.
"""

import numpy as np

B = 1
F = 3
T = 880
S = F * T
NH = 12
HD = 128
SCALE = 1.0 / np.sqrt(HD)
N_CORES = 8
UPC = 3
NKT = (S + 127) // 128
HALF_SPLITS = ((0, 512), (512, 368))


def build_program(reps: int = 1, loop_n: int = 1):
    from contextlib import ExitStack
    import concourse.tile as tile
    from concourse import bacc, mybir

    f32 = mybir.dt.float32
    f16 = mybir.dt.float16
    Exp = mybir.ActivationFunctionType.Exp

    nc = bacc.Bacc("TRN2", target_bir_lowering=False, debug=False,
                   num_devices=N_CORES)

    qT_d = nc.dram_tensor("qT", [UPC, HD, S], f16, kind="ExternalInput").ap()
    kT_d = nc.dram_tensor("kT", [UPC, HD, S], f16, kind="ExternalInput").ap()
    v_d = nc.dram_tensor("v", [UPC, 128, NKT, HD], f16, kind="ExternalInput").ap()
    ones_d = nc.dram_tensor("ones", [128, 128], f16, kind="ExternalInput").ap()
    o_d = nc.dram_tensor("o", [UPC, HD, S], f32, kind="ExternalOutput").ap()

    with ExitStack() as ctx:
        tc = ctx.enter_context(tile.TileContext(nc))
        const = ctx.enter_context(tc.tile_pool(name="const", bufs=1))
        big = ctx.enter_context(tc.tile_pool(name="big", bufs=2))
        pexp = ctx.enter_context(tc.tile_pool(name="pexp", bufs=6))
        lsum = ctx.enter_context(tc.tile_pool(name="lsum", bufs=2))
        outs = ctx.enter_context(tc.tile_pool(name="outs", bufs=2))
        psum = ctx.enter_context(tc.tile_pool(name="psum", bufs=1, space="PSUM"))

        ones = const.tile([128, 128], f16)
        nc.sync.dma_start(ones, ones_d)

        loop_ctx = tc.For_i(0, loop_n, 1) if loop_n > 1 else None
        if loop_ctx is not None:
            ctx.enter_context(loop_ctx)

        for _rep in range(reps):
            # One flat software pipeline across every (unit, frame, k-tile):
            # QK+exp run LOOKAHEAD steps ahead of PV so ACT never drains at
            # frame/unit boundaries. PSUM banks: acc(x2) 4 + ps(x2) 4 = 8;
            # the L-fold borrows a ps-ring slot at frame end.
            LOOKAHEAD = 2
            ustate = {}

            def load_unit(u):
                qs = big.tile([HD, S], f16, tag="q", name=f"q_{u}")
                ks = big.tile([HD, S], f16, tag="k", name=f"k_{u}")
                vs = big.tile([128, NKT, HD], f16, tag="v", name=f"v_{u}")
                # interleaved per-frame loads: frame-0 q/k land first so the
                # first QK starts early; v rides the (otherwise idle) Pool
                # engine's DMA queue to halve the serial stream
                for f in range(F):
                    cols = slice(f * T, (f + 1) * T)
                    nc.sync.dma_start(qs[:, cols], qT_d[u][:, cols])
                    nc.sync.dma_start(ks[:, cols], kT_d[u][:, cols])
                    c0, c1 = f * 7, min(NKT, (f + 1) * 7)
                    nc.sync.dma_start(vs[:, c0:c1], v_d[u][:, c0:c1])
                ustate[u] = (qs, ks, vs)

            fstate = {}

            def qk(u, f, kt):
                qs, ks, vs = ustate[u]
                kv = (f + 1) * T
                if kt == 0:
                    fstate[(u, f)] = (
                        psum.tile([128, T], f32, tag="acc", bufs=2,
                                  name=f"acc_{u}_{f}"),
                        [lsum.tile([128, T], f16, tag=f"ldve{i}",
                                   name=f"ldve{i}_{u}_{f}") for i in range(2)],
                    )
                k0 = kt * 128
                ksz = min(128, kv - k0)
                ps = psum.tile([128, T], f32, tag="ps", bufs=2,
                               name=f"ps_{u}_{f}_{kt}")
                for (h0, hw) in HALF_SPLITS:
                    nc.tensor.matmul(
                        ps[:ksz, h0:h0 + hw],
                        lhsT=ks[:, k0:k0 + ksz],
                        rhs=qs[:, f * T + h0:f * T + h0 + hw],
                        start=True, stop=True)
                pe = pexp.tile([128, T], f16, tag="pe",
                               name=f"pe_{u}_{f}_{kt}")
                nc.scalar.activation(pe[:ksz], ps[:ksz], Exp, scale=SCALE)
                return pe

            def pv(u, f, kt, pe):
                _, _, vs = ustate[u]
                kv = (f + 1) * T
                nkt = (kv + 127) // 128
                acc, ldve = fstate[(u, f)]
                ksz = min(128, kv - kt * 128)
                first, last = kt == 0, kt == nkt - 1
                for (h0, hw) in HALF_SPLITS:
                    nc.tensor.matmul(
                        acc[:, h0:h0 + hw],
                        lhsT=vs[:ksz, kt, :],
                        rhs=pe[:ksz, h0:h0 + hw],
                        start=first, stop=last)
                # fp16 row-sum partials, two interleaved accumulators
                if kt < 2:
                    nc.vector.tensor_copy(ldve[kt][:ksz], pe[:ksz])
                else:
                    a = ldve[kt % 2]
                    nc.vector.tensor_add(a[:ksz], a[:ksz], pe[:ksz])
                if not last:
                    return None
                # frame tail (emitted one pipeline step later so the next
                # QK precedes it in PE program order): fold fp16 partials
                # via ones-matmuls into replicated f32 L, normalize, store.
                def tail():
                    lfold = psum.tile([128, T], f32, tag="acc", bufs=2,
                                      name=f"lfold_{u}_{f}")
                    rec = outs.tile([128, T], f32, tag="rec")
                    o = outs.tile([128, T], f32, tag="o")
                    # per-half pipeline shortens the serial tail chain
                    for (h0, hw) in HALF_SPLITS:
                        h = slice(h0, h0 + hw)
                        for i in range(2):
                            nc.tensor.matmul(
                                lfold[:, h], lhsT=ones, rhs=ldve[i][:, h],
                                start=(i == 0), stop=(i == 1))
                        nc.vector.reciprocal(rec[:, h], lfold[:, h])
                        nc.vector.tensor_mul(o[:, h], acc[:, h], rec[:, h])
                        nc.sync.dma_start(
                            o_d[u][:, f * T + h0:f * T + h0 + hw], o[:, h])
                    del fstate[(u, f)]
                return tail

            stream = [(u, f, kt)
                      for u in range(UPC)
                      for f in range(F)
                      for kt in range(((f + 1) * T + 127) // 128)]
            pes = {}
            pending_tail = None
            for i in range(len(stream) + LOOKAHEAD):
                if i < len(stream):
                    u, f, kt = stream[i]
                    if f == 0 and kt == 0:
                        load_unit(u)
                    pes[i] = qk(u, f, kt)
                if pending_tail is not None:
                    pending_tail()
                    pending_tail = None
                j = i - LOOKAHEAD
                if j >= 0:
                    pending_tail = pv(*stream[j], pes.pop(j))
            if pending_tail is not None:
                pending_tail()

    nc.compile()
    return nc


# ------------------------------------------------------------------
# host-side transforms + packing (fp16 payloads)
# ------------------------------------------------------------------

def _rope_rotate(x, cos, sin):
    o = np.empty_like(x)
    x1 = x[:, 0::2]
    x2 = x[:, 1::2]
    o[:, 0::2] = x1 * cos - x2 * sin
    o[:, 1::2] = x2 * cos + x1 * sin
    return o


def _prope_mats(viewmats, Ks):
    P = np.zeros((F, 4, 4), np.float64)
    for f in range(F):
        Kpad = np.zeros((4, 4), np.float64)
        Kpad[:3, :3] = Ks[0, f].astype(np.float64)
        Kpad[3, 3] = 1.0
        P[f] = Kpad @ viewmats[0, f].astype(np.float64)
    Pinv = np.linalg.inv(P)
    return P, Pinv


def _apply_44(x, mats):
    o = np.empty_like(x)
    xc = x.reshape(F, T, HD // 4, 4)
    oc = o.reshape(F, T, HD // 4, 4)
    for f in range(F):
        oc[f] = (xc[f].astype(np.float64) @ mats[f]).astype(x.dtype)
    return o


def _pack_inputs(q, k, v, cos, sin, viewmats, Ks):
    q = np.asarray(q, np.float32)
    k = np.asarray(k, np.float32)
    v = np.asarray(v, np.float32)
    cos = np.asarray(cos, np.float32)
    sin = np.asarray(sin, np.float32)
    P, Pinv = _prope_mats(np.asarray(viewmats, np.float32),
                          np.asarray(Ks, np.float32))
    PT = np.ascontiguousarray(P.transpose(0, 2, 1))
    PinvT = np.ascontiguousarray(Pinv.transpose(0, 2, 1))

    in_maps = []
    for c in range(N_CORES):
        qTs, kTs, vss = [], [], []
        for j in range(UPC):
            u = c * UPC + j
            br, h = u // NH, u % NH
            qh = q[0, :, h, :]
            kh = k[0, :, h, :]
            vh = v[0, :, h, :]
            if br == 0:
                qh = _rope_rotate(qh, cos, sin)
                kh = _rope_rotate(kh, cos, sin)
            else:
                qh = _apply_44(qh, PT)
                kh = _apply_44(kh, Pinv)
                vh = _apply_44(vh, PinvT)
            qTs.append(np.ascontiguousarray(qh.T.astype(np.float16)))
            kTs.append(np.ascontiguousarray(kh.T.astype(np.float16)))
            vp = np.zeros((NKT * 128, HD), np.float16)
            vp[:S] = vh.astype(np.float16)
            vss.append(np.ascontiguousarray(
                vp.reshape(NKT, 128, HD).transpose(1, 0, 2)))
        in_maps.append({
            "qT": np.stack(qTs),
            "kT": np.stack(kTs),
            "v": np.stack(vss),
            "ones": np.ones((128, 128), np.float16),
        })
    return in_maps, P


def _unpack_outputs(results, P):
    rope = np.empty((B, S, NH, HD), np.float32)
    prope = np.empty((B, S, NH, HD), np.float32)
    PT = np.ascontiguousarray(P.transpose(0, 2, 1))
    for c in range(N_CORES):
        o = results[c]["o"]
        for j in range(UPC):
            u = c * UPC + j
            br, h = u // NH, u % NH
            oh = np.ascontiguousarray(o[j].T)
            if br == 0:
                rope[0, :, h, :] = oh
            else:
                prope[0, :, h, :] = _apply_44(oh, PT)
    return rope, prope


# ------------------------------------------------------------------
# execution (same runner machinery as baseline)
# ------------------------------------------------------------------

def make_runner(nc):
    """Build a jit-once callable: in_maps -> list[dict] of per-core outputs.

    Mirrors concourse.bass2jax.run_bass_via_pjrt's multi-core path, but
    hoists tracing/compilation out so repeated calls can be timed.
    """
    import jax
    import numpy as _np
    from jax.sharding import Mesh, PartitionSpec
    from jax.experimental.shard_map import shard_map
    from concourse import mybir
    from concourse import bass2jax

    bass2jax.install_neuronx_cc_hook()
    assert nc.dbg_addr is None or not nc.dbg_callbacks

    partition_name = (nc.partition_id_tensor.name
                      if nc.partition_id_tensor else None)
    in_names, out_names, out_avals, zero_outs = [], [], [], []
    for alloc in nc.m.functions[0].allocations:
        if not isinstance(alloc, mybir.MemoryLocationSet):
            continue
        name = alloc.memorylocations[0].name
        if alloc.kind == "ExternalInput":
            if name != partition_name:
                in_names.append(name)
        elif alloc.kind == "ExternalOutput":
            shape = tuple(alloc.tensor_shape)
            dtype = mybir.dt.np(alloc.dtype)
            out_names.append(name)
            out_avals.append(jax.core.ShapedArray(shape, dtype))
            zero_outs.append(_np.zeros(shape, dtype))
    n_params = len(in_names)
    n_outs = len(out_avals)
    all_in_names = list(in_names) + list(out_names)
    if partition_name is not None:
        all_in_names.append(partition_name)

    def _body(*args):
        operands = list(args)
        if partition_name is not None:
            operands.append(bass2jax.partition_id_tensor())
        outs = bass2jax._bass_exec_p.bind(
            *operands,
            out_avals=tuple(out_avals),
            in_names=tuple(all_in_names),
            out_names=tuple(out_names),
            lowering_input_output_aliases=(),
            sim_require_finite=True,
            sim_require_nnan=True,
            nc=nc,
        )
        return tuple(outs)

    devices = jax.devices()[:N_CORES]
    mesh = Mesh(_np.asarray(devices), ("core",))
    in_specs = (PartitionSpec("core"),) * (n_params + n_outs)
    out_specs = (PartitionSpec("core"),) * n_outs
    # No donation: our kernel writes every output element, so stale result
    # buffers are fine and the zero "outputs" can live on device across calls.
    sharded = jax.jit(
        shard_map(_body, mesh=mesh, in_specs=in_specs, out_specs=out_specs,
                  check_rep=False),
        keep_unused=True)
    sharding = jax.sharding.NamedSharding(mesh, PartitionSpec("core"))

    def stage(in_maps):
        """Transfer per-core inputs (+zero output operands) to device once."""
        concat_in = [
            _np.concatenate([_np.asarray(in_maps[c][nm]) for c in range(N_CORES)],
                            axis=0)
            for nm in in_names
        ]
        concat_zeros = [
            _np.zeros((N_CORES * z.shape[0], *z.shape[1:]), z.dtype)
            for z in zero_outs
        ]
        return [jax.device_put(a, sharding) for a in concat_in + concat_zeros]

    def run_staged(staged, want_outputs=True):
        out_arrs = sharded(*staged)
        jax.block_until_ready(out_arrs)
        if not want_outputs:
            return None
        return [
            {nm: _np.asarray(out_arrs[i]).reshape(N_CORES, *out_avals[i].shape)[c]
             for i, nm in enumerate(out_names)}
            for c in range(N_CORES)
        ]

    def run(in_maps, want_outputs=True):
        return run_staged(stage(in_maps), want_outputs)

    run.stage = stage
    run.run_staged = run_staged
    return run


_RUNNER_CACHE = {}
_PROGRAM_CACHE = {}


def _get_program(reps: int = 1, loop_n: int = 1):
    key = (reps, loop_n)
    if key not in _PROGRAM_CACHE:
        _PROGRAM_CACHE[key] = build_program(reps, loop_n)
    return _PROGRAM_CACHE[key]


def _get_runner(reps: int = 1, loop_n: int = 1):
    key = (reps, loop_n)
    if key not in _RUNNER_CACHE:
        _RUNNER_CACHE[key] = make_runner(_get_program(reps, loop_n))
    return _RUNNER_CACHE[key]


def kernel(q, k, v, cos, sin, viewmats, Ks):
    in_maps, P = _pack_inputs(q, k, v, cos, sin, viewmats, Ks)
    results = _get_runner(1)(in_maps)
    return _unpack_outputs(results, P)


# revision 5
# speedup vs baseline: 7.5330x; 1.6424x over previous
"""Trainium2 Bass kernel for CausalWanGameActionTransformerBlock.

Changes vs baseline:
- fp16 data path (q/k/v/p tiles): matmuls at 1 cyc/row, DVE element-wise
  ops hit the 2x two-byte mode, input DMA bytes halved.
- Row-sum L entirely on DVE (two interleaved fp16 accumulators, folded
  by a ones-matmul at frame end) — no streaming PE ones-matmuls, so PE
  does only QK+PV and PSUM fits acc + double-buffered score tiles.
"""

import numpy as np

B = 1
F = 3
T = 880
S = F * T
NH = 12
HD = 128
SCALE = 1.0 / np.sqrt(HD)
N_CORES = 8
UPC = 3
NKT = (S + 127) // 128
HALF_SPLITS = ((0, 512), (512, 368))


def build_program(reps: int = 1, loop_n: int = 1):
    from contextlib import ExitStack
    import concourse.tile as tile
    from concourse import bacc, mybir

    f32 = mybir.dt.float32
    f16 = mybir.dt.float16
    Exp = mybir.ActivationFunctionType.Exp

    nc = bacc.Bacc("TRN2", target_bir_lowering=False, debug=False,
                   num_devices=N_CORES)

    qT_d = nc.dram_tensor("qT", [UPC, HD, S], f16, kind="ExternalInput").ap()
    kT_d = nc.dram_tensor("kT", [UPC, HD, S], f16, kind="ExternalInput").ap()
    v_d = nc.dram_tensor("v", [UPC, 128, NKT, HD], f16, kind="ExternalInput").ap()
    ones_d = nc.dram_tensor("ones", [128, 128], f16, kind="ExternalInput").ap()
    o_d = nc.dram_tensor("o", [UPC, HD, S], f32, kind="ExternalOutput").ap()

    with ExitStack() as ctx:
        tc = ctx.enter_context(tile.TileContext(nc))
        const = ctx.enter_context(tc.tile_pool(name="const", bufs=1))
        big = ctx.enter_context(tc.tile_pool(name="big", bufs=2))
        pexp = ctx.enter_context(tc.tile_pool(name="pexp", bufs=6))
        lsum = ctx.enter_context(tc.tile_pool(name="lsum", bufs=2))
        outs = ctx.enter_context(tc.tile_pool(name="outs", bufs=2))
        psum = ctx.enter_context(tc.tile_pool(name="psum", bufs=1, space="PSUM"))

        ones = const.tile([128, 128], f16)
        nc.sync.dma_start(ones, ones_d)

        loop_ctx = tc.For_i(0, loop_n, 1) if loop_n > 1 else None
        if loop_ctx is not None:
            ctx.enter_context(loop_ctx)

        for _rep in range(reps):
            # One flat software pipeline across every (unit, frame, k-tile):
            # QK+exp run LOOKAHEAD steps ahead of PV so ACT never drains at
            # frame/unit boundaries. PSUM banks: acc(x2) 4 + ps(x2) 4 = 8;
            # the L-fold borrows a ps-ring slot at frame end.
            LOOKAHEAD = 2
            ustate = {}

            def load_unit(u):
                qs = big.tile([HD, S], f16, tag="q", name=f"q_{u}")
                ks = big.tile([HD, S], f16, tag="k", name=f"k_{u}")
                vs = big.tile([128, NKT, HD], f16, tag="v", name=f"v_{u}")
                # interleaved per-frame loads: frame-0 q/k land first so the
                # first QK starts early; v rides the (otherwise idle) Pool
                # engine's DMA queue to halve the serial stream
                for f in range(F):
                    cols = slice(f * T, (f + 1) * T)
                    nc.sync.dma_start(qs[:, cols], qT_d[u][:, cols])
                    nc.sync.dma_start(ks[:, cols], kT_d[u][:, cols])
                    c0, c1 = f * 7, min(NKT, (f + 1) * 7)
                    nc.sync.dma_start(vs[:, c0:c1], v_d[u][:, c0:c1])
                ustate[u] = (qs, ks, vs)

            fstate = {}

            def qk(u, f, kt):
                qs, ks, vs = ustate[u]
                kv = (f + 1) * T
                if kt == 0:
                    fstate[(u, f)] = (
                        psum.tile([128, T], f32, tag="acc", bufs=2,
                                  name=f"acc_{u}_{f}"),
                        [lsum.tile([128, T], f16, tag=f"ldve{i}",
                                   name=f"ldve{i}_{u}_{f}") for i in range(2)],
                    )
                k0 = kt * 128
                ksz = min(128, kv - k0)
                ps = psum.tile([128, T], f32, tag="ps", bufs=2,
                               name=f"ps_{u}_{f}_{kt}")
                for (h0, hw) in HALF_SPLITS:
                    nc.tensor.matmul(
                        ps[:ksz, h0:h0 + hw],
                        lhsT=ks[:, k0:k0 + ksz],
                        rhs=qs[:, f * T + h0:f * T + h0 + hw],
                        start=True, stop=True)
                pe = pexp.tile([128, T], f16, tag="pe",
                               name=f"pe_{u}_{f}_{kt}")
                nc.scalar.activation(pe[:ksz], ps[:ksz], Exp, scale=SCALE)
                return pe

            def pv(u, f, kt, pe):
                _, _, vs = ustate[u]
                kv = (f + 1) * T
                nkt = (kv + 127) // 128
                acc, ldve = fstate[(u, f)]
                ksz = min(128, kv - kt * 128)
                first, last = kt == 0, kt == nkt - 1
                for (h0, hw) in HALF_SPLITS:
                    nc.tensor.matmul(
                        acc[:, h0:h0 + hw],
                        lhsT=vs[:ksz, kt, :],
                        rhs=pe[:ksz, h0:h0 + hw],
                        start=first, stop=last)
                # fp16 row-sum partials, two interleaved accumulators
                if kt < 2:
                    nc.vector.tensor_copy(ldve[kt][:ksz], pe[:ksz])
                else:
                    a = ldve[kt % 2]
                    nc.vector.tensor_add(a[:ksz], a[:ksz], pe[:ksz])
                if not last:
                    return None
                # frame tail, split into one half-chain per later pipeline
                # step: each step inserts only ~400ns of PE fold work, which
                # the per-step PE slack absorbs without bubbling ACT.
                shared = {}

                def tail_half(h0, hw, first):
                    def run():
                        if first:
                            shared["lfold"] = psum.tile(
                                [128, T], f32, tag="acc", bufs=2,
                                name=f"lfold_{u}_{f}")
                            shared["rec"] = outs.tile([128, T], f32, tag="rec",
                                                      name=f"rec_{u}_{f}")
                            shared["o"] = outs.tile([128, T], f32, tag="o",
                                                    name=f"o_{u}_{f}")
                        lfold, rec, o = (shared["lfold"], shared["rec"],
                                         shared["o"])
                        h = slice(h0, h0 + hw)
                        for i in range(2):
                            nc.tensor.matmul(
                                lfold[:, h], lhsT=ones, rhs=ldve[i][:, h],
                                start=(i == 0), stop=(i == 1))
                        nc.vector.reciprocal(rec[:, h], lfold[:, h])
                        nc.vector.tensor_mul(o[:, h], acc[:, h], rec[:, h])
                        nc.sync.dma_start(
                            o_d[u][:, f * T + h0:f * T + h0 + hw], o[:, h])
                        if not first:
                            del fstate[(u, f)]
                    return run

                return [tail_half(h0, hw, n == 0)
                        for n, (h0, hw) in enumerate(HALF_SPLITS)]

            stream = [(u, f, kt)
                      for u in range(UPC)
                      for f in range(F)
                      for kt in range(((f + 1) * T + 127) // 128)]
            pes = {}
            pending = []
            for i in range(len(stream) + LOOKAHEAD):
                if i < len(stream):
                    u, f, kt = stream[i]
                    if f == 0 and kt == 0:
                        load_unit(u)
                    pes[i] = qk(u, f, kt)
                if pending:
                    pending.pop(0)()
                j = i - LOOKAHEAD
                if j >= 0:
                    t = pv(*stream[j], pes.pop(j))
                    if t:
                        pending.extend(t)
            while pending:
                pending.pop(0)()

    nc.compile()
    return nc


# ------------------------------------------------------------------
# host-side transforms + packing (fp16 payloads)
# ------------------------------------------------------------------

def _rope_rotate(x, cos, sin):
    o = np.empty_like(x)
    x1 = x[:, 0::2]
    x2 = x[:, 1::2]
    o[:, 0::2] = x1 * cos - x2 * sin
    o[:, 1::2] = x2 * cos + x1 * sin
    return o


def _prope_mats(viewmats, Ks):
    P = np.zeros((F, 4, 4), np.float64)
    for f in range(F):
        Kpad = np.zeros((4, 4), np.float64)
        Kpad[:3, :3] = Ks[0, f].astype(np.float64)
        Kpad[3, 3] = 1.0
        P[f] = Kpad @ viewmats[0, f].astype(np.float64)
    Pinv = np.linalg.inv(P)
    return P, Pinv


def _apply_44(x, mats):
    o = np.empty_like(x)
    xc = x.reshape(F, T, HD // 4, 4)
    oc = o.reshape(F, T, HD // 4, 4)
    for f in range(F):
        oc[f] = (xc[f].astype(np.float64) @ mats[f]).astype(x.dtype)
    return o


def _pack_inputs(q, k, v, cos, sin, viewmats, Ks):
    q = np.asarray(q, np.float32)
    k = np.asarray(k, np.float32)
    v = np.asarray(v, np.float32)
    cos = np.asarray(cos, np.float32)
    sin = np.asarray(sin, np.float32)
    P, Pinv = _prope_mats(np.asarray(viewmats, np.float32),
                          np.asarray(Ks, np.float32))
    PT = np.ascontiguousarray(P.transpose(0, 2, 1))
    PinvT = np.ascontiguousarray(Pinv.transpose(0, 2, 1))

    in_maps = []
    for c in range(N_CORES):
        qTs, kTs, vss = [], [], []
        for j in range(UPC):
            u = c * UPC + j
            br, h = u // NH, u % NH
            qh = q[0, :, h, :]
            kh = k[0, :, h, :]
            vh = v[0, :, h, :]
            if br == 0:
                qh = _rope_rotate(qh, cos, sin)
                kh = _rope_rotate(kh, cos, sin)
            else:
                qh = _apply_44(qh, PT)
                kh = _apply_44(kh, Pinv)
                vh = _apply_44(vh, PinvT)
            qTs.append(np.ascontiguousarray(qh.T.astype(np.float16)))
            kTs.append(np.ascontiguousarray(kh.T.astype(np.float16)))
            vp = np.zeros((NKT * 128, HD), np.float16)
            vp[:S] = vh.astype(np.float16)
            vss.append(np.ascontiguousarray(
                vp.reshape(NKT, 128, HD).transpose(1, 0, 2)))
        in_maps.append({
            "qT": np.stack(qTs),
            "kT": np.stack(kTs),
            "v": np.stack(vss),
            "ones": np.ones((128, 128), np.float16),
        })
    return in_maps, P


def _unpack_outputs(results, P):
    rope = np.empty((B, S, NH, HD), np.float32)
    prope = np.empty((B, S, NH, HD), np.float32)
    PT = np.ascontiguousarray(P.transpose(0, 2, 1))
    for c in range(N_CORES):
        o = results[c]["o"]
        for j in range(UPC):
            u = c * UPC + j
            br, h = u // NH, u % NH
            oh = np.ascontiguousarray(o[j].T)
            if br == 0:
                rope[0, :, h, :] = oh
            else:
                prope[0, :, h, :] = _apply_44(oh, PT)
    return rope, prope


# ------------------------------------------------------------------
# execution (same runner machinery as baseline)
# ------------------------------------------------------------------

def make_runner(nc):
    """Build a jit-once callable: in_maps -> list[dict] of per-core outputs.

    Mirrors concourse.bass2jax.run_bass_via_pjrt's multi-core path, but
    hoists tracing/compilation out so repeated calls can be timed.
    """
    import jax
    import numpy as _np
    from jax.sharding import Mesh, PartitionSpec
    from jax.experimental.shard_map import shard_map
    from concourse import mybir
    from concourse import bass2jax

    bass2jax.install_neuronx_cc_hook()
    assert nc.dbg_addr is None or not nc.dbg_callbacks

    partition_name = (nc.partition_id_tensor.name
                      if nc.partition_id_tensor else None)
    in_names, out_names, out_avals, zero_outs = [], [], [], []
    for alloc in nc.m.functions[0].allocations:
        if not isinstance(alloc, mybir.MemoryLocationSet):
            continue
        name = alloc.memorylocations[0].name
        if alloc.kind == "ExternalInput":
            if name != partition_name:
                in_names.append(name)
        elif alloc.kind == "ExternalOutput":
            shape = tuple(alloc.tensor_shape)
            dtype = mybir.dt.np(alloc.dtype)
            out_names.append(name)
            out_avals.append(jax.core.ShapedArray(shape, dtype))
            zero_outs.append(_np.zeros(shape, dtype))
    n_params = len(in_names)
    n_outs = len(out_avals)
    all_in_names = list(in_names) + list(out_names)
    if partition_name is not None:
        all_in_names.append(partition_name)

    def _body(*args):
        operands = list(args)
        if partition_name is not None:
            operands.append(bass2jax.partition_id_tensor())
        outs = bass2jax._bass_exec_p.bind(
            *operands,
            out_avals=tuple(out_avals),
            in_names=tuple(all_in_names),
            out_names=tuple(out_names),
            lowering_input_output_aliases=(),
            sim_require_finite=True,
            sim_require_nnan=True,
            nc=nc,
        )
        return tuple(outs)

    devices = jax.devices()[:N_CORES]
    mesh = Mesh(_np.asarray(devices), ("core",))
    in_specs = (PartitionSpec("core"),) * (n_params + n_outs)
    out_specs = (PartitionSpec("core"),) * n_outs
    # No donation: our kernel writes every output element, so stale result
    # buffers are fine and the zero "outputs" can live on device across calls.
    sharded = jax.jit(
        shard_map(_body, mesh=mesh, in_specs=in_specs, out_specs=out_specs,
                  check_rep=False),
        keep_unused=True)
    sharding = jax.sharding.NamedSharding(mesh, PartitionSpec("core"))

    def stage(in_maps):
        """Transfer per-core inputs (+zero output operands) to device once."""
        concat_in = [
            _np.concatenate([_np.asarray(in_maps[c][nm]) for c in range(N_CORES)],
                            axis=0)
            for nm in in_names
        ]
        concat_zeros = [
            _np.zeros((N_CORES * z.shape[0], *z.shape[1:]), z.dtype)
            for z in zero_outs
        ]
        return [jax.device_put(a, sharding) for a in concat_in + concat_zeros]

    def run_staged(staged, want_outputs=True):
        out_arrs = sharded(*staged)
        jax.block_until_ready(out_arrs)
        if not want_outputs:
            return None
        return [
            {nm: _np.asarray(out_arrs[i]).reshape(N_CORES, *out_avals[i].shape)[c]
             for i, nm in enumerate(out_names)}
            for c in range(N_CORES)
        ]

    def run(in_maps, want_outputs=True):
        return run_staged(stage(in_maps), want_outputs)

    run.stage = stage
    run.run_staged = run_staged
    return run


_RUNNER_CACHE = {}
_PROGRAM_CACHE = {}


def _get_program(reps: int = 1, loop_n: int = 1):
    key = (reps, loop_n)
    if key not in _PROGRAM_CACHE:
        _PROGRAM_CACHE[key] = build_program(reps, loop_n)
    return _PROGRAM_CACHE[key]


def _get_runner(reps: int = 1, loop_n: int = 1):
    key = (reps, loop_n)
    if key not in _RUNNER_CACHE:
        _RUNNER_CACHE[key] = make_runner(_get_program(reps, loop_n))
    return _RUNNER_CACHE[key]


def kernel(q, k, v, cos, sin, viewmats, Ks):
    in_maps, P = _pack_inputs(q, k, v, cos, sin, viewmats, Ks)
    results = _get_runner(1)(in_maps)
    return _unpack_outputs(results, P)
